# revision 1
# baseline (speedup 1.0000x reference)
"""Trainium2 Bass kernel for nn_BSplineKAN_44719199486017.

2-layer B-spline KAN on x[4, 4096, 512]. Data-parallel over 8 NeuronCores:
core c handles batch b=c//2, sequence half h=c%2 (2048 rows).

Math: the 4 cubic Cox-de Boor basis functions reduce exactly to
    N_j(u) = sum_k C[j,k] * relu(k-u)^3,    u = 517*(x-min)/(max-min)
so with m_k = min(u-k, 0) (so m_k^3 = -relu(k-u)^3) the spline matmul
becomes 4 plane matmuls with host-folded weights
    wl_k[f, o] = -sum_j C[j,k] * sw[o, 4f+j].

Per-(batch,feature) min/max over the full 4096-seq is combined across the
core pair with ONE 8-core AllReduce(max) over an [8, 512] buffer holding
(max, -min) rows per batch; each core places its stats in its own batch's
rows via a host-supplied 0/-inf mask (pure SPMD, no per-core constants).
"""
import numpy as np
from contextlib import ExitStack

import concourse.bass as bass
import concourse.tile as tile
import concourse.mybir as mybir
from concourse import bacc
from concourse.bass_utils import run_bass_kernel_spmd

F32 = mybir.dt.float32
F32R = mybir.dt.float32r
AF = mybir.ActivationFunctionType
OP = mybir.AluOpType
AX = mybir.AxisListType

B, S, F = 4, 4096, 512
SH = S // 2          # rows per core
NFT = F // 128       # feature tiles (4)
SC = 512             # s-chunk width for plane tiles
NSC = SH // SC       # chunks (4)
RPC = SC // 128      # row tiles per chunk (4)
N_CORES = 8
KNOT_SCALE = 517.0
EPS = 1e-5
NEG_INF = -3.0e38

BASIS_C = np.array([
    [1.0, 0.0, 0.0, 0.0],
    [-2.0, 0.25, 0.0, 0.0],
    [1.5, -0.75, 1.0 / 6.0, 0.0],
    [-2.0 / 3.0, 1.0, -2.0 / 3.0, 1.0 / 6.0],
], dtype=np.float64)  # [j, k-1]

_CACHE = {}


def _minmax_exchange(nc, dram, stat, src, cm, layer, dta_out):
    """Local min/max per feature tile + cross-pair combine via one 8-core
    AllReduce(max). Returns (xM, nxm) [128,1] tiles per feature tile.

    Layout: [2, B*F] — row 0 = per-batch maxes, row 1 = per-batch -mins,
    batches along the free dim (engine ops can't start at odd partitions).
    cm is a host [2, B*F] mask: 0 on my batch's segment, -inf elsewhere.
    """
    cc_in = dram.tile([2, B * F], F32, tag=f"cc_in{layer}", name=f"cc_in{layer}")
    cc_out = dram.tile([2, B * F], F32, tag=f"cc_out{layer}",
                       name=f"cc_out{layer}")
    scr = dram.tile([128, 2 * NFT], F32, tag=f"scr{layer}", name=f"scr{layer}")
    scr2 = dram.tile([2, F], F32, tag=f"scr2_{layer}", name=f"scr2_{layer}")

    packed = stat.tile([128, 2 * NFT], F32, tag="packed", name="packed", bufs=1)
    for ft in range(NFT):
        nc.vector.tensor_reduce(packed[:, 2 * ft:2 * ft + 1], src[ft][:],
                                axis=AX.X, op=OP.max)
        nc.vector.tensor_reduce(packed[:, 2 * ft + 1:2 * ft + 2], src[ft][:],
                                axis=AX.X, op=OP.min, negate=True)

    # row-ize via DRAM: scr[p, 2ft+r] -> (r, ft*128+p), replicated per batch
    nc.gpsimd.dma_start(scr[:], packed[:])
    cand = stat.tile([2, B * F], F32, tag="cand", name="cand", bufs=1)
    rowsrc = scr[:].rearrange("p (ft r) -> r ft p", r=2)
    for b in range(B):
        for r in range(2):
            nc.gpsimd.dma_start(cand[r:r + 1, b * F:(b + 1) * F], rowsrc[r])
    # mask: my batch segment + 0, others + (-inf)
    nc.vector.tensor_tensor(cand[:], cand[:], cm[:], op=OP.add)
    nc.gpsimd.dma_start(cc_in[:], cand[:])
    if getattr(nc, "_sim_mode", False):
        nc.gpsimd.dma_start(cc_out[:], cc_in[:])
    else:
        nc.gpsimd.collective_compute(
            "AllReduce", OP.max,
            ins=[cc_in.opt()], outs=[cc_out.opt()],
            replica_groups=[list(range(N_CORES))],
        )
    rob = stat.tile([2, B * F], F32, tag="rob", name="rob", bufs=1)
    nc.gpsimd.dma_start(rob[:], cc_out[:])
    nc.vector.tensor_tensor(rob[:], rob[:], cm[:], op=OP.add)
    ta = stat.tile([2, F], F32, tag="ta", name="ta", bufs=1)
    nc.vector.tensor_tensor(ta[:], rob[:, 0:F], rob[:, F:2 * F], op=OP.max)
    nc.vector.tensor_tensor(ta[:], ta[:], rob[:, 2 * F:3 * F], op=OP.max)
    nc.vector.tensor_tensor(ta[:], ta[:], rob[:, 3 * F:4 * F], op=OP.max)
    nc.gpsimd.dma_start(scr2[:], ta[:])
    nc.gpsimd.dma_start(dta_out[:][2 * layer:2 * layer + 2, :], ta[:])

    xM, nxm = [], []
    for ft in range(NFT):
        a = stat.tile([128, 1], F32, tag=f"xM{ft}", name=f"xM{ft}", bufs=1)
        nc.gpsimd.dma_start(a[:], scr2[:][0:1, ft * 128:(ft + 1) * 128]
                            .rearrange("one (p x) -> (one p) x", p=128))
        b_ = stat.tile([128, 1], F32, tag=f"nxm{ft}", name=f"nxm{ft}", bufs=1)
        nc.gpsimd.dma_start(b_[:], scr2[:][1:2, ft * 128:(ft + 1) * 128]
                            .rearrange("one (p x) -> (one p) x", p=128))
        xM.append(a)
        nxm.append(b_)
    return xM, nxm


def _build(sim=False, fast_gb=True):
    nc = bacc.Bacc("TRN2", target_bir_lowering=False, debug=False,
                   num_devices=1 if sim else N_CORES)
    nc._sim_mode = sim

    xT = nc.declare_dram_parameter("xT", [F, SH], F32, isOutput=False)
    W0 = nc.declare_dram_parameter("W0", [F, 5, F], F32R, isOutput=False)
    W1 = nc.declare_dram_parameter("W1", [F, 5, F], F32R, isOutput=False)
    GB0 = nc.declare_dram_parameter("GB0", [128, 2, F], F32, isOutput=False)
    GB1 = nc.declare_dram_parameter("GB1", [128, 2, F], F32, isOutput=False)
    CM = nc.declare_dram_parameter("CM", [2, B * F], F32, isOutput=False)
    EYE = nc.declare_dram_parameter("EYE", [128, 128], F32, isOutput=False)
    OUT = nc.declare_dram_parameter("out", [SH, F], F32, isOutput=True)
    DTA = nc.declare_dram_parameter("dta", [2 * 2, F], F32, isOutput=True)
    DH = nc.declare_dram_parameter("dh", [128, F], F32, isOutput=True)

    with ExitStack() as ctx:
        tc = ctx.enter_context(tile.TileContext(nc))
        dram = ctx.enter_context(tc.tile_pool(name="dram", bufs=1, space="DRAM"))
        wpool = ctx.enter_context(tc.tile_pool(name="w", bufs=1))
        xpool = ctx.enter_context(tc.tile_pool(name="x", bufs=1))
        hpool = ctx.enter_context(tc.tile_pool(name="h", bufs=1))
        lpool = ctx.enter_context(tc.tile_pool(name="l", bufs=2))
        stat = ctx.enter_context(tc.tile_pool(name="st", bufs=2))
        rpool = ctx.enter_context(tc.tile_pool(name="r", bufs=3))
        psum = ctx.enter_context(tc.tile_pool(name="ps", bufs=6, space="PSUM"))
        pstr = ctx.enter_context(tc.tile_pool(name="pstr", bufs=2, space="PSUM"))

        # ---- static loads -------------------------------------------------
        eye = wpool.tile([128, 128], F32)
        nc.sync.dma_start(eye[:], EYE[:])
        cm = wpool.tile([2, B * F], F32)
        nc.sync.dma_start(cm[:], CM[:])

        xts = []
        for ft in range(NFT):
            t = xpool.tile([128, SH], F32, tag=f"x{ft}", name=f"x{ft}")
            nc.sync.dma_start(t[:], xT.rearrange("(ft p) s -> ft p s", p=128)[ft])
            xts.append(t)

        h1T = [hpool.tile([128, SH], F32, tag=f"h1T{ft}", name=f"h1T{ft}")
               for ft in range(NFT)]

        for li in range(2):
            src = xts if li == 0 else h1T
            Wp = (W0, W1)[li]
            GBp = (GB0, GB1)[li]

            # per-layer weight loads (shared slots across layers)
            wt = []
            for ft in range(NFT):
                t = wpool.tile([128, 5, F], F32R, tag=f"w{ft}", name=f"w{ft}")
                nc.sync.dma_start(t[:], Wp.rearrange("(ft p) c n -> ft p c n",
                                                     p=128)[ft])
                wt.append(t)
            gbt = wpool.tile([128, 2, F], F32, tag="gb", name="gb")
            nc.sync.dma_start(gbt[:], GBp[:])

            xM, nxm = _minmax_exchange(nc, dram, stat, src, cm, li, DTA)

            # ---- per-feature affine scalars ------------------------------
            # u = su*x + sb ; su = 517/(xM + nxm); sb = su*nxm (= -su*xm)
            su, sb = [], []
            for ft in range(NFT):
                rng = stat.tile([128, 1], F32, tag=f"rng{ft}", name=f"rng{ft}",
                                bufs=1)
                nc.vector.tensor_tensor(rng[:], xM[ft][:], nxm[ft][:], op=OP.add)
                rcp = stat.tile([128, 1], F32, tag=f"rcp{ft}", name=f"rcp{ft}",
                                bufs=1)
                nc.vector.reciprocal(rcp[:], rng[:])
                s_ = stat.tile([128, 1], F32, tag=f"su{ft}", name=f"su{ft}",
                               bufs=1)
                nc.vector.tensor_scalar(s_[:], rcp[:], KNOT_SCALE, None,
                                        op0=OP.mult)
                b2 = stat.tile([128, 1], F32, tag=f"sb{ft}", name=f"sb{ft}",
                               bufs=1)
                nc.vector.tensor_tensor(b2[:], s_[:], nxm[ft][:], op=OP.mult)
                su.append(s_)
                sb.append(b2)

            # ---- per s-chunk: planes + matmuls + LN ----------------------
            for sc in range(NSC):
                s0 = sc * SC
                sl = slice(s0, s0 + SC)
                pss = [psum.tile([128, F], F32, tag="y", name="y")
                       for _ in range(RPC)]
                n_grp = 5 * NFT
                grp_i = 0

                # m4 per ft for this chunk (lives across the k loop)
                m4c = []
                for ft in range(NFT):
                    uc = lpool.tile([128, SC], F32, tag="uc", name="uc")
                    nc.vector.tensor_scalar(uc[:], src[ft][:, sl], su[ft][:],
                                            sb[ft][:], op0=OP.mult, op1=OP.add)
                    t4 = lpool.tile([128, SC], F32, tag=f"m4c{ft}",
                                    name=f"m4c{ft}")
                    nc.vector.tensor_scalar(t4[:], uc[:], 4.0, 0.0,
                                            op0=OP.subtract, op1=OP.min)
                    m4c.append(t4)

                def do_mms(plane, wslot):
                    nonlocal grp_i
                    for rc in range(RPC):
                        nc.tensor.matmul(pss[rc][:],
                                         plane[:, rc * 128:(rc + 1) * 128],
                                         wslot,
                                         start=(grp_i == 0),
                                         stop=(grp_i == n_grp - 1))
                    grp_i += 1

                # silu plane chunks + their matmuls
                for ft in range(NFT):
                    sil = lpool.tile([128, SC], F32R, tag="sil", name="sil",
                                     bufs=3)
                    nc.scalar.activation(sil[:], src[ft][:, sl], AF.Silu)
                    do_mms(sil[:], wt[ft][:, 0, :])

                # truncated-cube planes + their matmuls, k-major
                for k in (1, 2, 3, 4):
                    for ft in range(NFT):
                        if k == 4:
                            mk = m4c[ft][:]
                        else:
                            mt = lpool.tile([128, SC], F32, tag="mk",
                                            name="mk", bufs=3)
                            nc.vector.tensor_scalar(
                                mt[:], m4c[ft][:], float(4 - k), 0.0,
                                op0=OP.add, op1=OP.min)
                            mk = mt[:]
                        qt = lpool.tile([128, SC], F32, tag="qk", name="qk",
                                        bufs=3)
                        if k == 1:
                            nc.vector.tensor_tensor(qt[:], mk, mk, op=OP.mult)
                        else:
                            nc.scalar.activation(qt[:], mk, AF.Square)
                        lt = lpool.tile([128, SC], F32R, tag="lk", name="lk",
                                        bufs=4)
                        leng = nc.vector if (k + ft) % 2 else nc.gpsimd
                        leng.tensor_tensor(lt[:], qt[:], mk, op=OP.mult)
                        do_mms(lt[:], wt[ft][:, k, :])

                # ---- layernorm: batched scalar chain for the chunk -------
                stp = stat.tile([128, 2 * RPC], F32, tag="stp", name="stp")
                for rc in range(RPC):
                    st6 = stat.tile([128, 6], F32, tag="st6", name="st6")
                    nc.vector.bn_stats(st6[:], pss[rc][:])
                    nc.vector.bn_aggr(stp[:, 2 * rc:2 * rc + 2], st6[:])
                vep = stat.tile([128, RPC], F32, tag="vep", name="vep")
                # vars are odd columns of stp
                nc.vector.tensor_scalar(
                    vep[:], stp[:].rearrange("p (r two) -> p two r", two=2)[:, 1],
                    EPS, None, op0=OP.add)
                sdp = stat.tile([128, RPC], F32, tag="sdp", name="sdp")
                nc.scalar.activation(sdp[:], vep[:], AF.Sqrt)
                rsp = stat.tile([128, RPC], F32, tag="rsp", name="rsp")
                nc.vector.reciprocal(rsp[:], sdp[:])

                for rc in range(RPC):
                    r0 = s0 + rc * 128
                    ps = pss[rc]
                    t2 = rpool.tile([128, F], F32, tag="t2", name="t2")
                    nc.vector.tensor_scalar(t2[:], ps[:],
                                            stp[:, 2 * rc:2 * rc + 1],
                                            rsp[:, rc:rc + 1],
                                            op0=OP.subtract, op1=OP.mult)
                    if not fast_gb:
                        nc.vector.tensor_tensor(t2[:], t2[:], gbt[:, 0, :],
                                                op=OP.mult)
                        nc.gpsimd.tensor_tensor(t2[:], t2[:], gbt[:, 1, :],
                                                op=OP.add)

                    if li == 0:
                        hrow = rpool.tile([128, F], F32, tag="hrow",
                                          name="hrow", bufs=2)
                        nc.scalar.activation(hrow[:], t2[:], AF.Silu)
                        rt = sc * RPC + rc
                        if rt == 0:
                            nc.sync.dma_start(DH[:], hrow[:])
                        for ft in range(NFT):
                            pt = pstr.tile([128, 128], F32, tag="ptr",
                                           name="ptr")
                            nc.tensor.transpose(
                                pt[:], hrow[:, ft * 128:(ft + 1) * 128], eye[:])
                            if ft % 2 == 0:
                                nc.vector.tensor_copy(
                                    h1T[ft][:, rt * 128:(rt + 1) * 128], pt[:])
                            else:
                                nc.scalar.copy(
                                    h1T[ft][:, rt * 128:(rt + 1) * 128], pt[:])
                    else:
                        orow = rpool.tile([128, F], F32, tag="orow",
                                          name="orow", bufs=2)
                        nc.scalar.activation(orow[:], t2[:], AF.Silu)
                        nc.sync.dma_start(OUT[:][r0:r0 + 128, :], orow[:])

    nc.compile()
    return nc


def _prep_inputs(x, bw0, sw0, g0, b0, bw1, sw1, g1, b1):
    def fold(bw, sw):
        sw4 = np.asarray(sw, np.float64).reshape(F, F, 4)
        wl = -np.einsum('ofj,jk->kfo', sw4, BASIS_C)          # [4, f_in, o]
        W = np.empty((F, 5, F), np.float32)
        W[:, 0, :] = np.asarray(bw, np.float32).T
        for k in range(4):
            W[:, k + 1, :] = wl[k].astype(np.float32)
        return W

    def gbpack(g, b):
        GB = np.empty((128, 2, F), np.float32)
        GB[:, 0, :] = np.asarray(g, np.float32)[None, :]
        GB[:, 1, :] = np.asarray(b, np.float32)[None, :]
        return GB

    W0 = fold(bw0, sw0)
    W1 = fold(bw1, sw1)
    GB0 = gbpack(g0, b0)
    GB1 = gbpack(g1, b1)
    eye = np.eye(128, dtype=np.float32)

    in_maps = []
    for c in range(N_CORES):
        b_, h_ = divmod(c, 2)
        xs = np.ascontiguousarray(
            np.asarray(x, np.float32)[b_, h_ * SH:(h_ + 1) * SH, :].T)
        cmv = np.full((2, B * F), NEG_INF, np.float32)
        cmv[:, b_ * F:(b_ + 1) * F] = 0.0
        in_maps.append(dict(xT=xs, W0=W0, W1=W1, GB0=GB0, GB1=GB1,
                            CM=cmv, EYE=eye))
    return in_maps


def kernel(x, bw0, sw0, g0, b0, bw1, sw1, g1, b1):
    fast = (np.all(np.asarray(g0) == 1) and np.all(np.asarray(g1) == 1)
            and np.all(np.asarray(b0) == 0) and np.all(np.asarray(b1) == 0))
    key = "nc_fast" if fast else "nc_gen"
    if key not in _CACHE:
        _CACHE[key] = _build(fast_gb=fast)
    nc = _CACHE[key]
    in_maps = _prep_inputs(x, bw0, sw0, g0, b0, bw1, sw1, g1, b1)
    res = run_bass_kernel_spmd(nc, in_maps, list(range(N_CORES)))
    out = np.empty((B, S, F), np.float32)
    for c in range(N_CORES):
        b_, h_ = divmod(c, 2)
        out[b_, h_ * SH:(h_ + 1) * SH, :] = res.results[c]["out"]
    return out



# revision 5
# speedup vs baseline: 1.2098x; 1.2098x over previous
"""Trainium2 Bass kernel for nn_BSplineKAN_44719199486017.

2-layer B-spline KAN on x[4, 4096, 512]. Data-parallel over 8 NeuronCores:
core c handles batch b=c//2, sequence half h=c%2 (2048 rows).

Math: the 4 cubic Cox-de Boor basis functions reduce exactly to
    N_j(u) = sum_k C[j,k] * relu(k-u)^3,    u = 517*(x-min)/(max-min)
so the spline matmul becomes 4 plane matmuls with host-folded weights
    wk[f, o] = +sum_j C[j,k] * sw[o, 4f+j]
on planes r_k^3 with r_k = relu(k-u), built via a relu chain from
r_4 = relu(-su*x + (4-sb)) (one fused scalar-engine activation).

Per-(batch,feature) min/max over the full 4096-seq combines across the
core pair with a single pair-group AllReduce(max) on a [128, 8] stat
tile (max, -min per feature-partition) — features stay on partitions the
whole way, so no DRAM rearranges are needed.

Matmul planes and weights are fp16 (stationary gets the fast weight
load), accumulation stays fp32 in PSUM; h1 and the min/max path stay
fp32 (the spline basis is sensitive to min/max precision).
"""
import numpy as np
from contextlib import ExitStack

import concourse.bass as bass
import concourse.tile as tile
import concourse.mybir as mybir
from concourse import bacc
from concourse.bass_utils import run_bass_kernel_spmd

F32 = mybir.dt.float32
FP16 = mybir.dt.float16
AF = mybir.ActivationFunctionType
OP = mybir.AluOpType
AX = mybir.AxisListType

B, S, F = 4, 4096, 512
SH = S // 2          # rows per core
NFT = F // 128       # feature tiles (4)
PCW = 512            # plane-chunk width (rows per plane build)
NPC = SH // PCW      # plane chunks (4)
NG = SH // 128       # 128-row groups (16)
GPP = PCW // 128     # groups per plane chunk (4)
N_CORES = 8
KNOT_SCALE = 517.0
EPS = 1e-5
PAIR_GROUPS = [[0, 1], [2, 3], [4, 5], [6, 7]]

BASIS_C = np.array([
    [1.0, 0.0, 0.0, 0.0],
    [-2.0, 0.25, 0.0, 0.0],
    [1.5, -0.75, 1.0 / 6.0, 0.0],
    [-2.0 / 3.0, 1.0, -2.0 / 3.0, 1.0 / 6.0],
], dtype=np.float64)  # [j, k-1]

_CACHE = {}


def _build(sim=False, fast_gb=True):
    nc = bacc.Bacc("TRN2", target_bir_lowering=False, debug=False,
                   num_devices=1 if sim else N_CORES)
    nc._sim_mode = sim

    xT = nc.declare_dram_parameter("xT", [F, SH], F32, isOutput=False)
    W0 = nc.declare_dram_parameter("W0", [F, 5, F], FP16, isOutput=False)
    W1 = nc.declare_dram_parameter("W1", [F, 5, F], FP16, isOutput=False)
    GB0 = nc.declare_dram_parameter("GB0", [128, 2, F], F32, isOutput=False)
    GB1 = nc.declare_dram_parameter("GB1", [128, 2, F], F32, isOutput=False)
    EYE = nc.declare_dram_parameter("EYE", [128, 128], F32, isOutput=False)
    OUT = nc.declare_dram_parameter("out", [SH, F], F32, isOutput=True)

    with ExitStack() as ctx:
        tc = ctx.enter_context(tile.TileContext(nc))
        dram = ctx.enter_context(tc.tile_pool(name="dram", bufs=1, space="DRAM"))
        wpool = ctx.enter_context(tc.tile_pool(name="w", bufs=1))
        xpool = ctx.enter_context(tc.tile_pool(name="x", bufs=1))
        hpool = ctx.enter_context(tc.tile_pool(name="h", bufs=1))
        lpool = ctx.enter_context(tc.tile_pool(name="l", bufs=2))
        stat = ctx.enter_context(tc.tile_pool(name="st", bufs=1))
        rpool = ctx.enter_context(tc.tile_pool(name="r", bufs=2))
        psum = ctx.enter_context(tc.tile_pool(name="ps", bufs=6, space="PSUM"))
        pstr = ctx.enter_context(tc.tile_pool(name="pstr", bufs=2, space="PSUM"))

        # ---- input loads (x first: it gates the layer-0 stats) ----------
        xts = []
        for ft in range(NFT):
            t = xpool.tile([128, SH], F32, tag=f"x{ft}", name=f"x{ft}")
            nc.sync.dma_start(t[:], xT.rearrange("(ft p) s -> ft p s", p=128)[ft])
            xts.append(t)

        # ---- layer-0 min/max stats: reduce per half on vector -----------
        # pk cols: c*8 + r*4 + ft  (r: 0=max, 1=-min), c = half index
        pk0 = stat.tile([128, 16], F32, tag="pk0", name="pk0")
        for ft in range(NFT):
            for c in range(2):
                sl = slice(c * 1024, (c + 1) * 1024)
                nc.vector.tensor_reduce(pk0[:, c * 8 + ft:c * 8 + ft + 1],
                                        xts[ft][:, sl], axis=AX.X, op=OP.max)
                nc.vector.tensor_reduce(pk0[:, c * 8 + 4 + ft:c * 8 + 5 + ft],
                                        xts[ft][:, sl], axis=AX.X, op=OP.min,
                                        negate=True)

        def exchange(pk, layer):
            """pk [128, 16] half-stats -> (nsu, fb) [128, 4] for this layer."""
            st_ = stat.tile([128, 8], F32, tag=f"stx{layer}", name=f"stx{layer}")
            nc.vector.tensor_tensor(st_[:], pk[:, 0:8], pk[:, 8:16], op=OP.max)
            cc_in = dram.tile([128, 8], F32, tag=f"cc_in{layer}",
                              name=f"cc_in{layer}")
            cc_out = dram.tile([128, 8], F32, tag=f"cc_out{layer}",
                               name=f"cc_out{layer}")
            nc.gpsimd.dma_start(cc_in[:], st_[:])
            if getattr(nc, "_sim_mode", False):
                nc.gpsimd.dma_start(cc_out[:], cc_in[:])
            else:
                nc.gpsimd.collective_compute(
                    "AllReduce", OP.max,
                    ins=[cc_in.opt()], outs=[cc_out.opt()],
                    replica_groups=PAIR_GROUPS,
                )
            res = stat.tile([128, 8], F32, tag=f"res{layer}", name=f"res{layer}")
            nc.gpsimd.dma_start(res[:], cc_out[:])
            # su = 517/(xM - xm); sb = su*(-xm); fb = 4 - sb; nsu = -su
            rng = stat.tile([128, 4], F32, tag=f"rng{layer}", name=f"rng{layer}")
            nc.vector.tensor_tensor(rng[:], res[:, 0:4], res[:, 4:8], op=OP.add)
            rcp = stat.tile([128, 4], F32, tag=f"rcp{layer}", name=f"rcp{layer}")
            nc.vector.reciprocal(rcp[:], rng[:])
            su = stat.tile([128, 4], F32, tag=f"su{layer}", name=f"su{layer}")
            nc.vector.tensor_scalar(su[:], rcp[:], KNOT_SCALE, None, op0=OP.mult)
            nsu = stat.tile([128, 4], F32, tag=f"nsu{layer}", name=f"nsu{layer}")
            nc.vector.tensor_scalar(nsu[:], su[:], -1.0, None, op0=OP.mult)
            sb = stat.tile([128, 4], F32, tag=f"sb{layer}", name=f"sb{layer}")
            nc.vector.tensor_tensor(sb[:], su[:], res[:, 4:8], op=OP.mult)
            fb = stat.tile([128, 4], F32, tag=f"fb{layer}", name=f"fb{layer}")
            nc.vector.tensor_scalar(fb[:], sb[:], -1.0, 4.0,
                                    op0=OP.mult, op1=OP.add)
            return nsu, fb

        nsu0, fb0 = exchange(pk0, 0)

        # ---- weight loads (behind x on the sync queue) ------------------
        eye = wpool.tile([128, 128], F32)
        nc.sync.dma_start(eye[:], EYE[:])
        wts = []
        for li, Wp in enumerate((W0, W1)):
            wl = []
            for ft in range(NFT):
                t = wpool.tile([128, 5, F], FP16, tag=f"w{li}{ft}",
                               name=f"w{li}{ft}")
                nc.sync.dma_start(t[:], Wp.rearrange("(ft p) c n -> ft p c n",
                                                     p=128)[ft])
                wl.append(t)
            wts.append(wl)
        gbts = []
        if not fast_gb:
            for li, GBp in enumerate((GB0, GB1)):
                t = wpool.tile([128, 2, F], F32, tag=f"gb{li}", name=f"gb{li}")
                nc.sync.dma_start(t[:], GBp[:])
                gbts.append(t)

        h1T = [hpool.tile([128, SH], F32, tag=f"h1T{ft}", name=f"h1T{ft}")
               for ft in range(NFT)]
        pk1 = stat.tile([128, 16], F32, tag="pk1", name="pk1")

        stats = [(nsu0, fb0), None]

        for li in range(2):
            src = xts if li == 0 else h1T
            wt = wts[li]
            nsu, fb = stats[li]

            # base planes: silu(src) in bf16, full row width per ft
            sils = []
            for ft in range(NFT):
                t = lpool.tile([128, SH], FP16, tag=f"sil{ft}",
                               name=f"sil{ft}", bufs=1)
                nc.scalar.activation(t[:], src[ft][:], AF.Silu)
                sils.append(t)

            pss = {}

            def open_group(g, sils=sils, wt=wt, pss=pss):
                ps = psum.tile([128, F], F32, tag="y", name="y")
                pss[g] = ps
                for ft in range(NFT):
                    nc.tensor.matmul(ps[:],
                                     sils[ft][:, g * 128:(g + 1) * 128],
                                     wt[ft][:, 0, :],
                                     start=(ft == 0), stop=False)

            # pre-enqueue base matmuls for the first groups: they only
            # need sil + W and overlap the stats exchange on the PE queue
            for g in range(6):
                open_group(g)

            pend = None  # delayed transpose work (layer 0)

            def do_transposes(g, hrow):
                for ft in range(NFT):
                    pt = pstr.tile([128, 128], F32, tag="ptr", name="ptr")
                    nc.tensor.transpose(
                        pt[:], hrow[:, ft * 128:(ft + 1) * 128], eye[:])
                    if ft % 2 == 0:
                        nc.vector.tensor_copy(
                            h1T[ft][:, g * 128:(g + 1) * 128], pt[:])
                    else:
                        nc.scalar.copy(
                            h1T[ft][:, g * 128:(g + 1) * 128], pt[:])

            for pc in range(NPC):
                psl = slice(pc * PCW, (pc + 1) * PCW)
                # ---- spline planes r_k^3 for this chunk ------------------
                planes = [[None] * 5 for _ in range(NFT)]
                for ft in range(NFT):
                    r4 = lpool.tile([128, PCW], F32, tag="r4", name="r4",
                                    bufs=2)
                    nc.scalar.activation(r4[:], src[ft][:, psl], AF.Relu,
                                         bias=fb[:, ft:ft + 1],
                                         scale=nsu[:, ft:ft + 1])
                    rks = {4: r4}
                    for k in (3, 2, 1):
                        rk = lpool.tile([128, PCW], F32, tag="rk", name="rk",
                                        bufs=4)
                        nc.vector.tensor_scalar(rk[:], r4[:], float(4 - k),
                                                0.0, op0=OP.subtract,
                                                op1=OP.max)
                        rks[k] = rk
                    for k in (4, 3, 2, 1):
                        rk = rks[k]
                        qk = lpool.tile([128, PCW], F32, tag="qk", name="qk",
                                        bufs=3)
                        qeng = nc.vector if k >= 3 else nc.gpsimd
                        qeng.tensor_tensor(qk[:], rk[:], rk[:], op=OP.mult)
                        lk = lpool.tile([128, PCW], FP16, tag=f"lk{k}_{ft}",
                                        name=f"lk{k}_{ft}", bufs=2)
                        leng = nc.gpsimd if k >= 3 else nc.vector
                        leng.tensor_tensor(lk[:], qk[:], rk[:], op=OP.mult)
                        planes[ft][k] = lk

                # ---- row groups: spline matmuls + LN epilogue ------------
                for gg in range(GPP):
                    g = pc * GPP + gg
                    if g >= 6:
                        open_group(g)
                    ps = pss.pop(g)
                    n_sp = 4 * NFT
                    i_sp = 0
                    for k in (1, 2, 3, 4):
                        for ft in range(NFT):
                            i_sp += 1
                            nc.tensor.matmul(
                                ps[:],
                                planes[ft][k][:, gg * 128:(gg + 1) * 128],
                                wt[ft][:, k, :],
                                start=False, stop=(i_sp == n_sp))

                    # LayerNorm + silu
                    st6 = stat.tile([128, 6], F32, tag="st6", name="st6",
                                    bufs=2)
                    nc.vector.bn_stats(st6[:], ps[:])
                    stg = stat.tile([128, 2], F32, tag="stg", name="stg",
                                    bufs=2)
                    nc.vector.bn_aggr(stg[:], st6[:])
                    vep = stat.tile([128, 1], F32, tag="vep", name="vep",
                                    bufs=2)
                    nc.vector.tensor_scalar(vep[:], stg[:, 1:2], EPS, None,
                                            op0=OP.add)
                    sdp = stat.tile([128, 1], F32, tag="sdp", name="sdp",
                                    bufs=2)
                    nc.scalar.activation(sdp[:], vep[:], AF.Sqrt)
                    rsp = stat.tile([128, 1], F32, tag="rsp", name="rsp",
                                    bufs=2)
                    nc.vector.reciprocal(rsp[:], sdp[:])
                    t2 = rpool.tile([128, F], F32, tag="t2", name="t2",
                                    bufs=2)
                    nc.vector.tensor_scalar(t2[:], ps[:], stg[:, 0:1],
                                            rsp[:], op0=OP.subtract,
                                            op1=OP.mult)
                    if not fast_gb:
                        nc.vector.tensor_tensor(t2[:], t2[:],
                                                gbts[li][:, 0, :], op=OP.mult)
                        nc.gpsimd.tensor_tensor(t2[:], t2[:],
                                                gbts[li][:, 1, :], op=OP.add)

                    if li == 0:
                        hrow = rpool.tile([128, F], F32, tag="hrow",
                                          name="hrow", bufs=2)
                        nc.scalar.activation(hrow[:], t2[:], AF.Silu)
                        if pend is not None:
                            do_transposes(*pend)
                        pend = (g, hrow)
                    else:
                        orow = rpool.tile([128, F], F32, tag="orow",
                                          name="orow", bufs=2)
                        nc.scalar.activation(orow[:], t2[:], AF.Silu)
                        nc.sync.dma_start(OUT[:][g * 128:(g + 1) * 128, :],
                                          orow[:])

                # layer-1 stats: reduce finished h1T columns (1024-wide).
                # Flush the delayed transpose first — the reduce reads up to
                # the last group of this chunk.
                if li == 0 and pc % 2 == 1:
                    if pend is not None:
                        do_transposes(*pend)
                        pend = None
                    c = pc // 2
                    hsl = slice(c * 1024, (c + 1) * 1024)
                    for ft in range(NFT):
                        nc.vector.tensor_reduce(
                            pk1[:, c * 8 + ft:c * 8 + ft + 1],
                            h1T[ft][:, hsl], axis=AX.X, op=OP.max)
                        nc.vector.tensor_reduce(
                            pk1[:, c * 8 + 4 + ft:c * 8 + 5 + ft],
                            h1T[ft][:, hsl], axis=AX.X, op=OP.min,
                            negate=True)

            if li == 0:
                stats[1] = exchange(pk1, 1)

    nc.compile()
    return nc


def _prep_inputs(x, bw0, sw0, g0, b0, bw1, sw1, g1, b1):
    def fold(bw, sw):
        sw4 = np.asarray(sw, np.float64).reshape(F, F, 4)
        wk = np.einsum('ofj,jk->kfo', sw4, BASIS_C)           # [4, f_in, o]
        W = np.empty((F, 5, F), np.float32)
        W[:, 0, :] = np.asarray(bw, np.float32).T
        for k in range(4):
            W[:, k + 1, :] = wk[k].astype(np.float32)
        return W.astype(np.float16)

    def gbpack(g, b):
        GB = np.empty((128, 2, F), np.float32)
        GB[:, 0, :] = np.asarray(g, np.float32)[None, :]
        GB[:, 1, :] = np.asarray(b, np.float32)[None, :]
        return GB

    W0 = fold(bw0, sw0)
    W1 = fold(bw1, sw1)
    GB0 = gbpack(g0, b0)
    GB1 = gbpack(g1, b1)
    eye = np.eye(128, dtype=np.float32)

    in_maps = []
    for c in range(N_CORES):
        b_, h_ = divmod(c, 2)
        xs = np.ascontiguousarray(
            np.asarray(x, np.float32)[b_, h_ * SH:(h_ + 1) * SH, :].T)
        in_maps.append(dict(xT=xs, W0=W0, W1=W1, GB0=GB0, GB1=GB1, EYE=eye))
    return in_maps


def kernel(x, bw0, sw0, g0, b0, bw1, sw1, g1, b1):
    fast = (np.all(np.asarray(g0) == 1) and np.all(np.asarray(g1) == 1)
            and np.all(np.asarray(b0) == 0) and np.all(np.asarray(b1) == 0))
    key = "nc_fast" if fast else "nc_gen"
    if key not in _CACHE:
        _CACHE[key] = _build(fast_gb=fast)
    nc = _CACHE[key]
    in_maps = _prep_inputs(x, bw0, sw0, g0, b0, bw1, sw1, g1, b1)
    res = run_bass_kernel_spmd(nc, in_maps, list(range(N_CORES)))
    out = np.empty((B, S, F), np.float32)
    for c in range(N_CORES):
        b_, h_ = divmod(c, 2)
        out[b_, h_ * SH:(h_ + 1) * SH, :] = res.results[c]["out"]
    return out


# revision 9
# speedup vs baseline: 1.6392x; 1.3549x over previous
"""Trainium2 Bass kernel for nn_BSplineKAN_44719199486017.

2-layer B-spline KAN on x[4, 4096, 512]. Data-parallel over 8 NeuronCores:
core c handles batch b=c//2, sequence half h=c%2 (2048 rows).

Math: the 4 cubic Cox-de Boor basis functions reduce exactly to
    N_j(u) = sum_k C[j,k] * relu(k-u)^3,    u = 517*(x-min)/(max-min)
so the spline matmul becomes 4 plane matmuls with host-folded weights
    wk[f, o] = +sum_j C[j,k] * sw[o, 4f+j]
on planes r_k^3 with r_k = relu(k-u), built via a relu chain from
r_4 = relu(-su*x + (4-sb)) (one fused scalar-engine activation).

Per-(batch,feature) min/max over the full 4096-seq combines across the
core pair with a single pair-group AllReduce(max) on a [128, 8] stat
tile (max, -min per feature-partition) — features stay on partitions the
whole way, so no DRAM rearranges are needed.

Matmul planes and weights are fp16 (stationary gets the fast weight
load, 16-bit doubles DVE throughput), accumulation stays fp32 in PSUM;
h1 and the min/max path stay fp32 (the spline basis is sensitive to
min/max precision). Scalar-engine activations stick to one table set
(silu/relu/square) except one batched sqrt per 4-row-group chunk.
LayerNorm+silu is fused into a single PSUM-read activation per group:
silu(ps*rsig - mu*rsig).
"""
import numpy as np
from contextlib import ExitStack

import concourse.bass as bass
import concourse.tile as tile
import concourse.mybir as mybir
from concourse import bacc
from concourse.bass_utils import run_bass_kernel_spmd

F32 = mybir.dt.float32
F32R = mybir.dt.float32r
FP16 = mybir.dt.float16
AF = mybir.ActivationFunctionType
OP = mybir.AluOpType
AX = mybir.AxisListType

B, S, F = 4, 4096, 512
SH = S // 2          # rows per core
NFT = F // 128       # feature tiles (4)
PCW = 512            # plane-chunk width (rows per plane build)
NPC = SH // PCW      # plane chunks (4)
GPP = PCW // 128     # row groups per chunk (4)
N_CORES = 8
KNOT_SCALE = 517.0
EPS = 1e-5
PAIR_GROUPS = [[0, 1], [2, 3], [4, 5], [6, 7]]

BASIS_C = np.array([
    [1.0, 0.0, 0.0, 0.0],
    [-2.0, 0.25, 0.0, 0.0],
    [1.5, -0.75, 1.0 / 6.0, 0.0],
    [-2.0 / 3.0, 1.0, -2.0 / 3.0, 1.0 / 6.0],
], dtype=np.float64)  # [j, k-1]

_CACHE = {}


def _build(sim=False, fast_gb=True):
    nc = bacc.Bacc("TRN2", target_bir_lowering=False, debug=False,
                   num_devices=1 if sim else N_CORES)
    nc._sim_mode = sim

    xT = nc.declare_dram_parameter("xT", [F, SH], F32, isOutput=False)
    W0 = nc.declare_dram_parameter("W0", [F, 5, F], FP16, isOutput=False)
    W1 = nc.declare_dram_parameter("W1", [F, 5, F], FP16, isOutput=False)
    GB0 = nc.declare_dram_parameter("GB0", [128, 2, F], F32, isOutput=False)
    GB1 = nc.declare_dram_parameter("GB1", [128, 2, F], F32, isOutput=False)
    EYE = nc.declare_dram_parameter("EYE", [128, 128], F32, isOutput=False)
    OUT = nc.declare_dram_parameter("out", [SH, F], F32, isOutput=True)

    with ExitStack() as ctx:
        tc = ctx.enter_context(tile.TileContext(nc))
        dram = ctx.enter_context(tc.tile_pool(name="dram", bufs=1, space="DRAM"))
        wpool = ctx.enter_context(tc.tile_pool(name="w", bufs=1))
        xpool = ctx.enter_context(tc.tile_pool(name="x", bufs=1))
        hpool = ctx.enter_context(tc.tile_pool(name="h", bufs=1))
        lpool = ctx.enter_context(tc.tile_pool(name="l", bufs=2))
        stat = ctx.enter_context(tc.tile_pool(name="st", bufs=1))
        rpool = ctx.enter_context(tc.tile_pool(name="r", bufs=2))
        psum = ctx.enter_context(tc.tile_pool(name="ps", bufs=6, space="PSUM"))
        pstr = ctx.enter_context(tc.tile_pool(name="pstr", bufs=2, space="PSUM"))

        # ---- input loads (x first: it gates the layer-0 stats) ----------
        xts = []
        for ft in range(NFT):
            t = xpool.tile([128, SH], F32, tag=f"x{ft}", name=f"x{ft}")
            nc.sync.dma_start(t[:], xT.rearrange("(ft p) s -> ft p s", p=128)[ft])
            xts.append(t)

        # ---- layer-0 min/max stats: reduce per half on vector -----------
        # pk cols: c*8 + r*4 + ft  (r: 0=max, 1=-min), c = half index
        pk0 = stat.tile([128, 16], F32, tag="pk0", name="pk0")
        for ft in range(NFT):
            for c in range(2):
                sl = slice(c * 1024, (c + 1) * 1024)
                nc.vector.tensor_reduce(pk0[:, c * 8 + ft:c * 8 + ft + 1],
                                        xts[ft][:, sl], axis=AX.X, op=OP.max)
                nc.vector.tensor_reduce(pk0[:, c * 8 + 4 + ft:c * 8 + 5 + ft],
                                        xts[ft][:, sl], axis=AX.X, op=OP.min,
                                        negate=True)

        def exchange(pk, layer):
            """pk [128, 16] half-stats -> (nsu, fb) [128, 4] for this layer."""
            st_ = stat.tile([128, 8], F32, tag=f"stx{layer}", name=f"stx{layer}")
            nc.vector.tensor_tensor(st_[:], pk[:, 0:8], pk[:, 8:16], op=OP.max)
            cc_in = dram.tile([128, 8], F32, tag=f"cc_in{layer}",
                              name=f"cc_in{layer}")
            cc_out = dram.tile([128, 8], F32, tag=f"cc_out{layer}",
                               name=f"cc_out{layer}")
            nc.gpsimd.dma_start(cc_in[:], st_[:])
            if getattr(nc, "_sim_mode", False):
                nc.gpsimd.dma_start(cc_out[:], cc_in[:])
            else:
                nc.gpsimd.collective_compute(
                    "AllReduce", OP.max,
                    ins=[cc_in.opt()], outs=[cc_out.opt()],
                    replica_groups=PAIR_GROUPS,
                )
            res = stat.tile([128, 8], F32, tag=f"res{layer}", name=f"res{layer}")
            nc.gpsimd.dma_start(res[:], cc_out[:])
            # su = 517/(xM - xm); sb = su*(-xm); fb = 4 - sb; nsu = -su
            rng = stat.tile([128, 4], F32, tag=f"rng{layer}", name=f"rng{layer}")
            nc.vector.tensor_tensor(rng[:], res[:, 0:4], res[:, 4:8], op=OP.add)
            rcp = stat.tile([128, 4], F32, tag=f"rcp{layer}", name=f"rcp{layer}")
            nc.vector.reciprocal(rcp[:], rng[:])
            su = stat.tile([128, 4], F32, tag=f"su{layer}", name=f"su{layer}")
            nc.vector.tensor_scalar(su[:], rcp[:], KNOT_SCALE, None, op0=OP.mult)
            nsu = stat.tile([128, 4], F32, tag=f"nsu{layer}", name=f"nsu{layer}")
            nc.vector.tensor_scalar(nsu[:], su[:], -1.0, None, op0=OP.mult)
            sb = stat.tile([128, 4], F32, tag=f"sb{layer}", name=f"sb{layer}")
            nc.vector.tensor_tensor(sb[:], su[:], res[:, 4:8], op=OP.mult)
            fb = stat.tile([128, 4], F32, tag=f"fb{layer}", name=f"fb{layer}")
            nc.vector.tensor_scalar(fb[:], sb[:], -1.0, 4.0,
                                    op0=OP.mult, op1=OP.add)
            return nsu, fb

        nsu0, fb0 = exchange(pk0, 0)

        # ---- weight loads (behind x on the sync queue) ------------------
        eye = wpool.tile([128, 128], F32)
        nc.sync.dma_start(eye[:], EYE[:])
        wts = []
        for li, Wp in enumerate((W0, W1)):
            wl = []
            for ft in range(NFT):
                t = wpool.tile([128, 5, F], FP16, tag=f"w{li}{ft}",
                               name=f"w{li}{ft}")
                nc.sync.dma_start(t[:], Wp.rearrange("(ft p) c n -> ft p c n",
                                                     p=128)[ft])
                wl.append(t)
            wts.append(wl)
        gbts = []
        if not fast_gb:
            for li, GBp in enumerate((GB0, GB1)):
                t = wpool.tile([128, 2, F], F32, tag=f"gb{li}", name=f"gb{li}")
                nc.sync.dma_start(t[:], GBp[:])
                gbts.append(t)

        h1T = [hpool.tile([128, SH], F32, tag=f"h1T{ft}", name=f"h1T{ft}")
               for ft in range(NFT)]
        pk1 = stat.tile([128, 16], F32, tag="pk1", name="pk1")

        # base planes silu(src), fp16. Layer 0: full width now. Layer 1:
        # tiles created here, filled incrementally as h1T columns land.
        sils = [[None] * NFT for _ in range(2)]
        for ft in range(NFT):
            t = lpool.tile([128, SH], FP16, tag=f"sil0_{ft}",
                           name=f"sil0_{ft}", bufs=1)
            nc.scalar.activation(t[:], xts[ft][:], AF.Silu)
            sils[0][ft] = t
            sils[1][ft] = lpool.tile([128, SH], FP16, tag=f"sil1_{ft}",
                                     name=f"sil1_{ft}", bufs=1)

        pss = {}

        def open_group(li, g):
            ps = psum.tile([128, F], F32, tag="y", name="y")
            pss[(li, g)] = ps
            for ft in range(NFT):
                nc.tensor.matmul(ps[:],
                                 sils[li][ft][:, g * 128:(g + 1) * 128],
                                 wts[li][ft][:, 0, :],
                                 start=(ft == 0), stop=False)

        for g in range(6):
            open_group(0, g)

        stats = [(nsu0, fb0), None]
        pend = None  # delayed transpose work (layer 0)

        def do_transposes(g, hrow):
            for ft in range(NFT):
                pt = pstr.tile([128, 128], F32, tag="ptr", name="ptr")
                nc.tensor.transpose(
                    pt[:], hrow[:, ft * 128:(ft + 1) * 128], eye[:])
                if ft % 2 == 0:
                    nc.vector.tensor_copy(
                        h1T[ft][:, g * 128:(g + 1) * 128], pt[:])
                else:
                    nc.scalar.copy(
                        h1T[ft][:, g * 128:(g + 1) * 128], pt[:])

        for li in range(2):
            src = xts if li == 0 else h1T
            wt = wts[li]
            nsu, fb = stats[li]

            for pc in range(NPC):
                psl = slice(pc * PCW, (pc + 1) * PCW)
                # ---- spline planes r_k^3 (fp16) for this chunk -----------
                planes = [[None] * 5 for _ in range(NFT)]
                for ft in range(NFT):
                    r4 = lpool.tile([128, PCW], FP16, tag="r4", name="r4",
                                    bufs=2)
                    nc.scalar.activation(r4[:], src[ft][:, psl], AF.Relu,
                                         bias=fb[:, ft:ft + 1],
                                         scale=nsu[:, ft:ft + 1])
                    rks = {4: r4}
                    for k in (3, 2, 1):
                        rk = lpool.tile([128, PCW], FP16, tag="rk", name="rk",
                                        bufs=4)
                        nc.vector.tensor_scalar(rk[:], r4[:], float(4 - k),
                                                0.0, op0=OP.subtract,
                                                op1=OP.max)
                        rks[k] = rk
                    for k in (4, 3, 2, 1):
                        rk = rks[k]
                        qk = lpool.tile([128, PCW], FP16, tag="qk", name="qk",
                                        bufs=3)
                        if k == 4:
                            nc.scalar.activation(qk[:], rk[:], AF.Square)
                        elif k == 3:
                            nc.vector.tensor_tensor(qk[:], rk[:], rk[:],
                                                    op=OP.mult)
                        else:
                            nc.gpsimd.tensor_tensor(qk[:], rk[:], rk[:],
                                                    op=OP.mult)
                        lk = lpool.tile([128, PCW], FP16, tag=f"lk{k}_{ft}",
                                        name=f"lk{k}_{ft}", bufs=2)
                        nc.vector.tensor_tensor(lk[:], qk[:], rk[:],
                                                op=OP.mult)
                        planes[ft][k] = lk

                # ---- row groups: spline matmuls + batched LN -------------
                stp = stat.tile([128, 2, GPP], F32, tag="stp", name="stp",
                                bufs=2)
                gps = []
                for gg in range(GPP):
                    g = pc * GPP + gg
                    if (li, g) not in pss:
                        open_group(li, g)
                    ps = pss.pop((li, g))
                    gps.append(ps)
                    n_sp = 4 * NFT
                    i_sp = 0
                    for k in (1, 2, 3, 4):
                        for ft in range(NFT):
                            i_sp += 1
                            nc.tensor.matmul(
                                ps[:],
                                planes[ft][k][:, gg * 128:(gg + 1) * 128],
                                wt[ft][:, k, :],
                                start=False, stop=(i_sp == n_sp))
                    st6 = stat.tile([128, 6], F32, tag="st6", name="st6",
                                    bufs=2)
                    nc.vector.bn_stats(st6[:], ps[:])
                    nc.vector.bn_aggr(stp[:, :, gg], st6[:])

                # rsig for the chunk: one table-switch pair per chunk
                vep = stat.tile([128, GPP], F32, tag="vep", name="vep",
                                bufs=2)
                nc.vector.tensor_scalar(vep[:], stp[:, 1, :], EPS, None,
                                        op0=OP.add)
                sdp = stat.tile([128, GPP], F32, tag="sdp", name="sdp",
                                bufs=2)
                nc.scalar.activation(sdp[:], vep[:], AF.Sqrt)
                rsp = stat.tile([128, GPP], F32, tag="rsp", name="rsp",
                                bufs=2)
                nc.vector.reciprocal(rsp[:], sdp[:])
                # nmr = -mu * rsig
                nmr = stat.tile([128, GPP], F32, tag="nmr", name="nmr",
                                bufs=2)
                nc.vector.scalar_tensor_tensor(nmr[:], stp[:, 0, :], -1.0,
                                               rsp[:], op0=OP.mult,
                                               op1=OP.mult)

                for gg in range(GPP):
                    g = pc * GPP + gg
                    ps = gps[gg]
                    if fast_gb:
                        row = rpool.tile([128, F], F32, tag="row",
                                         name="row", bufs=3)
                        nc.scalar.activation(row[:], ps[:], AF.Silu,
                                             bias=nmr[:, gg:gg + 1],
                                             scale=rsp[:, gg:gg + 1])
                    else:
                        t2 = rpool.tile([128, F], F32, tag="t2", name="t2",
                                        bufs=2)
                        nc.vector.tensor_scalar(t2[:], ps[:],
                                                stp[:, 0, gg:gg + 1],
                                                rsp[:, gg:gg + 1],
                                                op0=OP.subtract, op1=OP.mult)
                        nc.vector.tensor_tensor(t2[:], t2[:],
                                                gbts[li][:, 0, :], op=OP.mult)
                        nc.gpsimd.tensor_tensor(t2[:], t2[:],
                                                gbts[li][:, 1, :], op=OP.add)
                        row = rpool.tile([128, F], F32, tag="row",
                                         name="row", bufs=3)
                        nc.scalar.activation(row[:], t2[:], AF.Silu)

                    if li == 0:
                        if pend is not None:
                            do_transposes(*pend)
                        pend = (g, row)
                    else:
                        nc.sync.dma_start(OUT[:][g * 128:(g + 1) * 128, :],
                                          row[:])

                # layer-0 tail work per completed 1024 columns of h1T:
                # flush transposes, fill layer-1 sil, reduce layer-1 stats
                if li == 0 and pc % 2 == 1:
                    if pend is not None:
                        do_transposes(*pend)
                        pend = None
                    c = pc // 2
                    hsl = slice(c * 1024, (c + 1) * 1024)
                    for ft in range(NFT):
                        nc.scalar.activation(sils[1][ft][:, hsl],
                                             h1T[ft][:, hsl], AF.Silu)
                        nc.vector.tensor_reduce(
                            pk1[:, c * 8 + ft:c * 8 + ft + 1],
                            h1T[ft][:, hsl], axis=AX.X, op=OP.max)
                        nc.vector.tensor_reduce(
                            pk1[:, c * 8 + 4 + ft:c * 8 + 5 + ft],
                            h1T[ft][:, hsl], axis=AX.X, op=OP.min,
                            negate=True)

            if li == 0:
                stats[1] = exchange(pk1, 1)
                # overlap the exchange with layer-1 base matmuls
                for g in range(6):
                    open_group(1, g)

    nc.compile()
    return nc


def _prep_inputs(x, bw0, sw0, g0, b0, bw1, sw1, g1, b1):
    def fold(bw, sw):
        sw4 = np.asarray(sw, np.float64).reshape(F, F, 4)
        wk = np.einsum('ofj,jk->kfo', sw4, BASIS_C)           # [4, f_in, o]
        W = np.empty((F, 5, F), np.float32)
        W[:, 0, :] = np.asarray(bw, np.float32).T
        for k in range(4):
            W[:, k + 1, :] = wk[k].astype(np.float32)
        return W.astype(np.float16)

    def gbpack(g, b):
        GB = np.empty((128, 2, F), np.float32)
        GB[:, 0, :] = np.asarray(g, np.float32)[None, :]
        GB[:, 1, :] = np.asarray(b, np.float32)[None, :]
        return GB

    W0 = fold(bw0, sw0)
    W1 = fold(bw1, sw1)
    GB0 = gbpack(g0, b0)
    GB1 = gbpack(g1, b1)
    eye = np.eye(128, dtype=np.float32)

    in_maps = []
    for c in range(N_CORES):
        b_, h_ = divmod(c, 2)
        xs = np.ascontiguousarray(
            np.asarray(x, np.float32)[b_, h_ * SH:(h_ + 1) * SH, :].T)
        in_maps.append(dict(xT=xs, W0=W0, W1=W1, GB0=GB0, GB1=GB1, EYE=eye))
    return in_maps


def kernel(x, bw0, sw0, g0, b0, bw1, sw1, g1, b1):
    fast = (np.all(np.asarray(g0) == 1) and np.all(np.asarray(g1) == 1)
            and np.all(np.asarray(b0) == 0) and np.all(np.asarray(b1) == 0))
    key = "nc_fast" if fast else "nc_gen"
    if key not in _CACHE:
        _CACHE[key] = _build(fast_gb=fast)
    nc = _CACHE[key]
    in_maps = _prep_inputs(x, bw0, sw0, g0, b0, bw1, sw1, g1, b1)
    res = run_bass_kernel_spmd(nc, in_maps, list(range(N_CORES)))
    out = np.empty((B, S, F), np.float32)
    for c in range(N_CORES):
        b_, h_ = divmod(c, 2)
        out[b_, h_ * SH:(h_ + 1) * SH, :] = res.results[c]["out"]
    return out
